# revision 71
# baseline (speedup 1.0000x reference)
"""Multi-head causal attention (B=2, T=2048, D=1024, H=16, HD=64) on 8 TRN2
NeuronCores.

Sharding: batch x head-group. Core c handles batch c//4 and heads
[4*(c%4), 4*(c%4)+4). Wq/Wk/Wv are split column-wise, Wo row-wise; each core
produces a full [T, D] partial output (its 4 heads' contribution, after
per-head softmax normalization and its Wo row-block), which the host sums
across the 4 cores of each batch and adds the bias to.

Per-core kernel (all matmuls contract along SBUF partitions; operands bf16,
accumulation fp32 in PSUM):
  xT [D, T] bf16 (host passes x[b].T pre-cast), weights bf16.
  QT/KT computed transposed [2*64hd, T] per head-pair (lhsT = w, rhs = xT).
  V computed natural [T, 4*64hd] (lhsT = xT, rhs = wv), stored bf16 with a
  ones-column per head (stride 66) so the P@V matmul also produces the
  softmax row-sums (M = 65).
  Scores are computed transposed, ST[k, q] (lhsT = KT, rhs = QT), exact-causal
  (q >= 128*kt per k-tile), exp'd on ACT (scale=1/8 fused) to bf16 ET tiles;
  the strictly-lower triangle of the leading 128x128 diagonal block is zeroed
  with a multiplicative mask (gpsimd).

  The two heads of a group sit at partition halves 0-63 / 64-127 of QT/KT, so
  their K=64 score matmuls land in different PE row-groups and execute
  CONCURRENTLY when interleaved back-to-back (PE row tiling): the schedule is
  pair-serial (pair 0 = heads 0,1; pair 1 = heads 2,3), with both heads'
  score/exp/context work interleaved per k-tile slot.

  CT' = V'.T @ ET accumulates per-q-chunk groups in PSUM ([65, 512] for
  chunks 0-2, [65, 256] halves "3a"/"3b" for the last 512 queries so the
  final chains start two slots earlier); partition 64 is the softmax
  denominator (exact fp32). Normalization: reshape-bounce through DRAM to
  [8, 64], DVE reciprocal, partition-broadcast read to [64, qw], DVE/gpsimd
  multiply into packed bf16 CT_g [128c, T] tiles. Pair 1's tail chunks
  instead compute the reciprocal as exp(-ln(d)) on the (by then idle) ACT
  engine with a single DRAM hop.
  out[t, do] = CT_g.T @ wo (fp16 partial); the output projection is dribbled
  into pair-1's window as q-chunks complete; each [128, 512] half DMAs on a
  different ring (gpsimd/sync, scalar at the tail).

Emission keeps PE dense (HAM warm): QT/KT(g0) upfront (first ST sub-tile
after 3 units); V + most QT/KT(g1) units spread through pair-0's window,
whose last two slots also pre-compute pair-1's kt 0-1 first score sub-tiles;
the remaining g1 units fill pair-1's first slots; per-slot early-dribble (CT
chunks whose exps finished in previous slots run before the slot's STs);
keep-warm dummy matmuls bridge the tail's normalization-chain latency.
"""

import contextlib

import numpy as np

T, D = 2048, 1024
NH, HD = 16, 64
HPC = 4  # heads per core
NCORES = 8
ND = D // 128  # 8 d-tiles
NT = T // 128  # 16 t/k-tiles
NQ = T // 512  # 4 q-chunks

_NC = None


def _build_nc():
    import concourse.mybir as mybir
    import concourse.tile as tile
    from concourse import bacc
    from concourse.masks import make_upper_triangular

    f32 = mybir.dt.float32
    bf16 = mybir.dt.bfloat16
    fp16 = mybir.dt.float16
    Exp = mybir.ActivationFunctionType.Exp

    nc = bacc.Bacc("TRN2", target_bir_lowering=False, debug=False, num_devices=NCORES)

    xT_d = nc.dram_tensor("xT", [D, T], bf16, kind="ExternalInput").ap()
    wq_d = nc.dram_tensor("wq", [D, HPC * HD], bf16, kind="ExternalInput").ap()
    wk_d = nc.dram_tensor("wk", [D, HPC * HD], bf16, kind="ExternalInput").ap()
    wv_d = nc.dram_tensor("wv", [D, HPC * HD], bf16, kind="ExternalInput").ap()
    wo_d = nc.dram_tensor("wo", [HPC * HD, D], bf16, kind="ExternalInput").ap()
    out_d = nc.dram_tensor("out", [T, D], fp16, kind="ExternalOutput").ap()
    rscr = nc.dram_tensor("rscr", [160, 64], f32).ap()
    rscr2 = nc.dram_tensor("rscr2", [160, 64], f32).ap()

    with tile.TileContext(nc) as tc, contextlib.ExitStack() as ctx:
        pool = lambda **kw: ctx.enter_context(tc.tile_pool(**kw))
        constp = pool(name="const", bufs=1)
        qkp = pool(name="qk", bufs=1)
        vp = pool(name="vpool", bufs=1)
        wop = pool(name="wop", bufs=1)
        etp = pool(name="et", bufs=2)
        # kt 0-3 ET tiles need 4 buffers: pair 1's score tiles for these
        # k-tiles are computed at the end of window 0, while pair 0's are
        # still being consumed by its j=3 context groups.
        etp01 = pool(name="et01", bufs=4)
        stgp = pool(name="stg", bufs=3)
        ctgp = pool(name="ctg", bufs=1)
        normp = pool(name="norm", bufs=4)
        rbp = pool(name="rb", bufs=3)
        ohp = pool(name="oh", bufs=3)
        rcpp = pool(name="rcpt", bufs=4)
        bctx = contextlib.ExitStack()
        psST = bctx.enter_context(tc.tile_pool(name="psST", bufs=2, space="PSUM"))
        psCT = bctx.enter_context(tc.tile_pool(name="psCT", bufs=2, space="PSUM"))
        actx = contextlib.ExitStack()
        apool = lambda **kw: actx.enter_context(tc.tile_pool(**kw))
        xtp = apool(name="xtr", bufs=1)
        wtp = apool(name="wtiles", bufs=1)
        psProj = apool(name="psProj", bufs=2, space="PSUM")
        octx = contextlib.ExitStack()

        mask = constp.tile([128, 128], bf16, name="mask")
        make_upper_triangular(nc, mask[:], val=1.0, diag=True)

        # Warm-up: the HAM clock gate only reaches 2.4GHz after ~3.4us of
        # sustained PE activity, and the first ~17us of projection units
        # otherwise run at the cold 1.2GHz while also waiting on input DMAs.
        # A dummy matmul stream on the mask constant (no DMA dependency)
        # spans the load window so the real units start warm. The 1-wide exp
        # pulls the ~1.3us ACT table load into the same window.
        wst = psST.tile([128, 128], f32, name="warmup", tag="st")
        for k in range(30):
            nc.tensor.matmul(wst[:], mask[:], mask[:], start=(k == 0), stop=(k == 29))
        wact = constp.tile([128, 1], f32, name="wact")
        nc.scalar.activation(wact[:], mask[:, 0:1], Exp, scale=1.0)

        QT = [qkp.tile([128, T], bf16, name=f"QT{g}") for g in range(2)]
        KT = [qkp.tile([128, T], bf16, name=f"KT{g}") for g in range(2)]
        vsb = [vp.tile([128, 66 * HPC], bf16, name=f"v{tt}") for tt in range(NT)]
        wo_sb = [wop.tile([128, D], bf16, name=f"wo{gi}") for gi in range(2)]

        # ---------- loads (bf16 straight from DRAM, split across DMA queues) ----
        wtiles = {}
        wsb = {}

        def load_w(wname, wd, eng):
            wsb[wname] = wtp.tile([128, ND * 256], bf16, name=f"{wname}sb", tag=f"{wname}sb")
            eng.dma_start(wsb[wname][:], wd.rearrange("(a p) c -> p a c", p=128))
            wtiles[wname] = [wsb[wname][:, 256 * dt : 256 * (dt + 1)] for dt in range(ND)]

        load_w("wq", wq_d, nc.sync)
        xtr = [xtp.tile([128, T], bf16, name=f"xtr{dt}", tag=f"xtr{dt}") for dt in range(ND)]
        # x columns in quarter-chunks, both halves of each quarter on
        # different DMA rings so the first quarter (all the first QT unit
        # needs besides wq) lands in ~half the serial-descriptor time.
        for q4 in range(4):
            for dt in range(ND):
                eng = nc.scalar if dt < 4 else nc.gpsimd
                eng.dma_start(
                    xtr[dt][:, 512 * q4 : 512 * (q4 + 1)],
                    xT_d[128 * dt : 128 * (dt + 1), 512 * q4 : 512 * (q4 + 1)],
                )
        load_w("wk", wk_d, nc.sync)
        load_w("wv", wv_d, nc.sync)
        for gi in range(2):
            nc.sync.dma_start(wo_sb[gi][:], wo_d[128 * gi : 128 * (gi + 1), :])

        # ---------- emission units ----------
        def emit_qkt_unit(wname, outs, g, c):
            ps = psProj.tile([128, 512], f32, name=f"pj_{wname}{g}_{c}", tag="proj")
            for dt in range(ND):
                nc.tensor.matmul(
                    ps[:],
                    wtiles[wname][dt][:, 128 * g : 128 * (g + 1)],
                    xtr[dt][:, 512 * c : 512 * (c + 1)],
                    start=(dt == 0),
                    stop=(dt == ND - 1),
                )
            nc.vector.tensor_copy(outs[g][:, 512 * c : 512 * (c + 1)], ps[:])

        def emit_v(tt):
            ps = psProj.tile([128, 256], f32, name=f"vps{tt}", tag="proj")
            for dt in range(ND):
                nc.tensor.matmul(
                    ps[:],
                    xtr[dt][:, 128 * tt : 128 * (tt + 1)],
                    wtiles["wv"][dt][:],
                    start=(dt == 0),
                    stop=(dt == ND - 1),
                )
            nc.any.memset(vsb[tt][:, 64 : 66 * HPC : 66], 1.0)
            for h in range(HPC):
                nc.vector.tensor_copy(vsb[tt][:, 66 * h : 66 * h + 64], ps[:, 64 * h : 64 * (h + 1)])

        ets = {}  # (h, kt) -> ET tile

        def emit_st_pair(g, kt, subs=None):
            # Both heads of pair g, interleaved at matmul level so the K=64
            # score matmuls alternate PE row groups (0-63 / 64-127) and run
            # concurrently. `subs` restricts to specific 1024-wide q sub-tiles
            # (used to split kt 0-1 of pair 1 across the window boundary).
            w = T - 128 * kt
            hs = (2 * g, 2 * g + 1)
            if subs is None:
                subs = range((w + 1023) // 1024)
            if 0 in subs:
                ep = etp01 if kt < 2 else etp
                for h in hs:
                    ets[(h, kt)] = ep.tile([128, w], bf16, name=f"et_h{h}_kt{kt}", tag=f"et{kt}")
            for sub in subs:
                sw = min(1024, w - 1024 * sub)
                q0 = 128 * kt + 1024 * sub
                pss = {
                    h: psST.tile([128, sw], f32, name=f"st_h{h}_k{kt}_s{sub}", tag="st")
                    for h in hs
                }
                for c in range((sw + 511) // 512):
                    n = min(512, sw - 512 * c)
                    for h in hs:
                        p0 = 64 * (h % 2)
                        nc.tensor.matmul(
                            pss[h][:, 512 * c : 512 * c + n],
                            KT[g][p0 : p0 + 64, 128 * kt : 128 * (kt + 1)],
                            QT[g][p0 : p0 + 64, q0 + 512 * c : q0 + 512 * c + n],
                            start=True,
                            stop=True,
                        )
                for h in hs:
                    et = ets[(h, kt)]
                    nc.scalar.activation(
                        et[:, 1024 * sub : 1024 * sub + sw], pss[h][:, 0:sw], Exp, scale=0.125
                    )
                    if sub == 0:
                        nc.gpsimd.tensor_mul(et[:, 0:128], et[:, 0:128], mask[:])

        stg = {}
        ct_ps = {}

        # CT groups: q-chunks 0-2 are 512 wide; the last 512 queries are
        # split into two 256-wide groups "3a"/"3b" so 3a (k-tiles 0-13) can
        # stop two slots earlier and its normalization + output projection
        # overlap the end of the window instead of trailing it.
        CT_QR = {0: (0, 512), 1: (512, 512), 2: (1024, 512), "3a": (1536, 256), "3b": (1792, 256)}
        CT_IDX = {0: 0, 1: 1, 2: 2, "3a": 3}  # + 4*h; "3b" -> 16 + h

        def _norm_idx(h, j):
            return 16 + h if j == "3b" else 4 * h + CT_IDX[j]

        def emit_ct_mms(h, j, kts, first, last):
            qb, qw = CT_QR[j]
            if first:
                ct_ps[(h, j)] = psCT.tile([65, qw], f32, name=f"ct_h{h}_j{j}", tag="ct")
            ct = ct_ps[(h, j)]
            for kt in kts:
                etoff = qb - 128 * kt
                if etoff >= 0:
                    n, psoff, ecol = min(qw, T - 128 * kt - etoff), 0, etoff
                else:
                    n, psoff, ecol = qw + etoff, -etoff, 0
                nc.tensor.matmul(
                    ct[0:65, psoff : psoff + n],
                    vsb[kt][:, 66 * h : 66 * h + 65],
                    ets[(h, kt)][:, ecol : ecol + n],
                    start=(kt == 0),
                    stop=(last and kt == kts[-1]),
                )

        def finish_ct(h, j):
            qb, qw = CT_QR[j]
            ct = ct_ps[(h, j)]
            s = stgp.tile([65, qw], f32, name=f"stg_h{h}_j{j}", tag="stg")
            stg[(h, j)] = s
            nc.vector.tensor_copy(s[:], ct[:])
            if not (j in ("3a", "3b") and h >= 2):  # pair-1 tail: ACT path, no bounce
                idx = _norm_idx(h, j)
                nc.sync.dma_start(rscr[8 * idx : 8 * idx + qw // 64, :], s[64:65, :])

        CTG = [ctgp.tile([128, T], bf16, name=f"ctg{gi}") for gi in range(2)]
        rscr2v = rscr2.rearrange("(r p) c -> r (p c)", p=8)  # [20, 512] view

        def emit_norm(h, j):
            g, half = h // 2, h % 2
            qb, qw = CT_QR[j]
            idx = _norm_idx(h, j)
            rb = rbp.tile([64, qw], f32, name=f"rb{idx}", tag="rb")
            if j in ("3a", "3b") and h >= 2:
                # pair-1 tail chunks: the ACT engine is idle by then —
                # reciprocal via exp(-ln(d)) on two narrow ACT ops straight
                # from stg, then an SBUF->SBUF partition-broadcast DMA. Skips
                # both DRAM bounces of the mid-kernel path (~2.5us shorter).
                lg = rcpp.tile([1, qw], f32, name=f"lg{idx}", tag="lg")
                nc.scalar.activation(
                    lg[:], stg[(h, j)][64:65, 0:qw], mybir.ActivationFunctionType.Ln
                )
                rt = rcpp.tile([1, qw], f32, name=f"rt{idx}", tag="rt")
                nc.scalar.activation(
                    rt[:], lg[:], mybir.ActivationFunctionType.Exp, scale=-1.0
                )
                # one DRAM hop for the partition broadcast (SBUF sources can't
                # take a stride-0 partition dim)
                nc.sync.dma_start(rscr2[8 * idx : 8 * idx + qw // 64, :], rt[:])
                nc.sync.dma_start(
                    rb[:], rscr2v[idx : idx + 1, 0:qw].partition_broadcast(64)
                )
                eng = nc.vector if h % 2 == 0 else nc.gpsimd
            else:
                # reciprocal of the row-sums via a DRAM-bounce reshape to
                # [8, 64] (a [1, 512] single-partition DVE reciprocal measures
                # ~3.3us), then a partition-broadcast DRAM read to [64, qw].
                r = qw // 64
                rs_hj = normp.tile([r, 64], f32, name=f"rs{idx}", tag="rs")
                nc.sync.dma_start(rs_hj[:], rscr[8 * idx : 8 * idx + r, :])
                rc_hj = normp.tile([r, 64], f32, name=f"rc{idx}", tag="rc")
                nc.vector.reciprocal(rc_hj[:], rs_hj[:])
                nc.sync.dma_start(rscr2[8 * idx : 8 * idx + r, :], rc_hj[:])
                nc.sync.dma_start(
                    rb[:], rscr2v[idx : idx + 1, 0:qw].partition_broadcast(64)
                )
                eng = nc.vector if idx % 2 == 0 else nc.gpsimd
            eng.tensor_mul(
                CTG[g][64 * half : 64 * half + 64, qb : qb + qw],
                stg[(h, j)][0:64, :],
                rb[:],
            )

        psO = None

        def emit_warm(n):
            # Keep-warm dummy matmuls: consume any long-ready SBUF data into a
            # dead psST-ring tile nobody reads. Woven into the dependency-
            # stall-prone tail so the PE's HAM activity window never sees a
            # >3.4us idle stretch (which would halve the clock for ~everything
            # that follows).
            wt = psST.tile([128, 512], f32, name=f"warm{emit_warm.i}", tag="st")
            emit_warm.i += 1
            for k in range(n):
                nc.tensor.matmul(
                    wt[:],
                    wo_sb[0][:, 0:128],
                    QT[0][:, 512 * (k % 4) : 512 * (k % 4) + 512],
                    start=(k == 0),
                    stop=(k == n - 1),
                )

        emit_warm.i = 0

        def emit_oproj(tt, eng, eng2=None, q2=None):
            # Each [128, 512] half is copied and DMA'd independently; the two
            # DMAs split across the gpsimd and sync DMA rings (scalar at the
            # tail, once the exp stream is done) so the 4MB of output drains
            # in parallel with compute.
            oh = ohp.tile([128, D], fp16, name=f"oh{tt}", tag="oh")
            for dc, ceng, q in ((0, eng, nc.gpsimd), (1, eng2 or eng, q2 or nc.sync)):
                ps = psO.tile([128, 512], f32, name=f"op{tt}_{dc}", tag="ops")
                for gi in range(2):
                    nc.tensor.matmul(
                        ps[:],
                        CTG[gi][:, 128 * tt : 128 * (tt + 1)],
                        wo_sb[gi][:, 512 * dc : 512 * (dc + 1)],
                        start=(gi == 0),
                        stop=(gi == 1),
                    )
                copy = ceng.copy if ceng is nc.scalar else ceng.tensor_copy
                copy(oh[:, 512 * dc : 512 * (dc + 1)], ps[:])
                q.dma_start(
                    out_d[128 * tt : 128 * (tt + 1), 512 * dc : 512 * (dc + 1)],
                    oh[:, 512 * dc : 512 * (dc + 1)],
                )

        # ---------- schedule ----------
        # Prologue: the first ST sub-tile (q 0..1023) only needs QT(g0) c0-1
        # and KT(g0) c0 — start the exp stream after three units instead of
        # five (the pre-warm units run at the cold 1.2 GHz clock).
        emit_qkt_unit("wq", QT, 0, 0)
        emit_qkt_unit("wq", QT, 0, 1)
        emit_qkt_unit("wk", KT, 0, 0)
        emit_st_pair(0, 0, subs=[0])
        emit_qkt_unit("wq", QT, 0, 2)
        emit_qkt_unit("wq", QT, 0, 3)

        # CT dribble tables: each head has exactly ONE live CT PSUM group at a
        # time (psCT bufs=2 covers the pair). Stops are pulled as early as
        # k-tile exp availability allows so normalization and the output
        # projection chase the window instead of trailing it. Pair 1's k-tiles
        # 0-1 are exp'd at the end of window 0, so its j0 stops at slot 3.
        def mkdrib(spec):
            t = {sw: [] for sw in range(NT)}
            for j, parts in spec:
                for i, (sl, lo, hi) in enumerate(parts):
                    t[sl].append((j, list(range(lo, hi)), i == 0, i == len(parts) - 1))
            return t

        drib0 = mkdrib(
            [
                (0, [(1, 0, 1), (2, 1, 2), (3, 2, 3), (4, 3, 4)]),
                (1, [(5, 0, 2), (6, 2, 4), (7, 4, 6), (8, 6, 8)]),
                (2, [(9, 0, 6), (10, 6, 10), (11, 10, 12)]),
                ("3a", [(12, 0, 7), (13, 7, 14)]),
                ("3b", [(14, 0, 8), (15, 8, 16)]),
            ]
        )
        drib1 = mkdrib(
            [
                (0, [(1, 0, 2), (2, 2, 3), (3, 3, 4)]),
                (1, [(4, 0, 2), (5, 2, 4), (6, 4, 6), (8, 6, 8)]),
                (2, [(9, 0, 6), (10, 6, 10), (11, 10, 12)]),
                ("3a", [(12, 0, 7), (13, 7, 14)]),
                ("3b", [(14, 0, 8), (15, 8, 16)]),
            ]
        )

        def emit_dribble(dribble, sw, hs, early=None):
            # early=True: only chunks whose k-tiles were exp'd in earlier
            # slots (emitted BEFORE the slot's ST pair, so the CT matmuls
            # never wait on this slot's exp); early=False: the rest.
            for j, kts_, first, last in dribble[sw]:
                if early is not None and (max(kts_) < sw) != early:
                    continue
                for h in hs:
                    emit_ct_mms(h, j, kts_, first, last)
                    if last:
                        finish_ct(h, j)
                        emit_norm(h, j)

        # pair-0 window: ST pair + V units + KT(g0) c1-3 + three g1 units +
        # CT dribble (heads 0,1). The last two slots pre-compute pair-1's
        # kt 0-1 first score sub-tiles so window 1's CT can start immediately
        # and its ACT stream is ~4us lighter.
        for sw in range(NT):
            emit_dribble(drib0, sw, (0, 1), early=True)
            emit_st_pair(0, sw, subs=[1] if sw == 0 else None)
            emit_v(sw)
            if 1 <= sw <= 3:
                emit_qkt_unit("wk", KT, 0, sw)
            elif sw == 5:
                emit_qkt_unit("wq", QT, 1, 0)
            elif sw == 7:
                emit_qkt_unit("wq", QT, 1, 1)
            elif sw == 9:
                emit_qkt_unit("wk", KT, 1, 0)
            elif sw == 11:
                # c2 must land in window 0: pair-1's kt1 sub-0 (q 128..1151)
                # reads QT[1] columns into chunk c2.
                emit_qkt_unit("wq", QT, 1, 2)
            elif sw >= 14:
                # pre-compute pair-1's kt 0-1 first score sub-tiles: balances
                # window 1's ACT load and lets its CT start immediately.
                emit_st_pair(1, sw - 14, subs=[0])
            emit_dribble(drib0, sw, (0, 1), early=False)

        # pair-1 window: remaining g1 projection units fill the early slots
        # (each lands just before the first ST slot that consumes it), ST +
        # CT dribble (heads 2,3), and the out-projection dribbled in as
        # q-chunks complete (~3 slots of norm DMA latency after each stop).
        oproj_sched = {sw: [] for sw in range(NT)}
        oproj_sched[5] = [0]
        oproj_sched[6] = [1]
        oproj_sched[7] = [2]
        oproj_sched[8] = [3]
        oproj_sched[11] = [4, 5]
        oproj_sched[12] = [6, 7]
        oproj_sched[14] = [8, 9]
        oproj_sched[15] = [10, 11]
        for sw in range(NT):
            if sw == 0:
                emit_qkt_unit("wq", QT, 1, 3)
                emit_st_pair(1, 0, subs=[1])
                emit_st_pair(1, 1, subs=[1])
            elif sw == 1:
                emit_qkt_unit("wk", KT, 1, 1)
                emit_st_pair(1, 2)
            elif sw == 2:
                emit_qkt_unit("wk", KT, 1, 2)
                emit_st_pair(1, 3)
            elif sw == 3:
                emit_qkt_unit("wk", KT, 1, 3)
            emit_dribble(drib1, sw, (2, 3), early=True)
            if sw >= 4:
                emit_st_pair(1, sw)
            emit_dribble(drib1, sw, (2, 3), early=False)
            if sw >= 12:
                emit_warm(3)
            for tt in oproj_sched[sw]:
                emit_oproj(tt, nc.vector, eng2=nc.scalar if sw >= 14 else None)
            if sw == 3:
                actx.close()
                psO = octx.enter_context(tc.tile_pool(name="psO", bufs=2, space="PSUM"))

        # ---------- output projection tail (t-tiles of q 1536-2047) ----------
        # dummy blocks keep the PE's HAM activity up across the j3a/j3b
        # normalization chains so the trailing projections run at full clock
        for tt in range(12, NT):
            emit_warm(3)
            emit_oproj(tt, nc.vector, eng2=nc.scalar, q2=nc.scalar)
        emit_warm(2)
        octx.close()
        bctx.close()

    nc.compile()
    return nc


def _get_nc():
    global _NC
    if _NC is None:
        _NC = _build_nc()
    return _NC


def make_in_maps(x, wq, wk, wv, wo):
    import ml_dtypes

    bf = ml_dtypes.bfloat16
    in_maps = []
    for c in range(NCORES):
        b, g4 = c // 4, c % 4
        cs = slice(256 * g4, 256 * (g4 + 1))
        in_maps.append(
            {
                "xT": np.ascontiguousarray(x[b].T).astype(bf),
                "wq": np.ascontiguousarray(wq[:, cs]).astype(bf),
                "wk": np.ascontiguousarray(wk[:, cs]).astype(bf),
                "wv": np.ascontiguousarray(wv[:, cs]).astype(bf),
                "wo": np.ascontiguousarray(wo[cs, :]).astype(bf),
            }
        )
    return in_maps


def kernel(x, wq, wk, wv, wo, bo):
    from concourse.bass_utils import run_bass_kernel_spmd

    x = np.asarray(x, dtype=np.float32)
    wq = np.asarray(wq, dtype=np.float32)
    wk = np.asarray(wk, dtype=np.float32)
    wv = np.asarray(wv, dtype=np.float32)
    wo = np.asarray(wo, dtype=np.float32)
    bo = np.asarray(bo, dtype=np.float32)

    nc = _get_nc()
    in_maps = make_in_maps(x, wq, wk, wv, wo)
    try:
        res = run_bass_kernel_spmd(nc, in_maps, core_ids=list(range(NCORES))).results
    except Exception:
        # transient NRT device errors have been observed once after a fresh
        # compile; one retry recovers
        res = run_bass_kernel_spmd(nc, in_maps, core_ids=list(range(NCORES))).results
    out = np.zeros((2, T, D), dtype=np.float32)
    for c in range(NCORES):
        out[c // 4] += res[c]["out"].astype(np.float32)
    out += bo[None, None, :]
    return out
